# revision 1
# baseline (speedup 1.0000x reference)
"""NetTGCN forward pass on 8 Trainium2 NeuronCores (Bass/Tile).

Sharding:
  Layer 1 (ChebTimeConv on the 4096-node graph): 4-way node-shard x 2-way
  batch-shard. The dense normalized adjacency (x2, transposed, bf16) stays
  resident in SBUF; each Chebyshev iteration all-gathers the new state
  across the 4 node shards (4-rank groups). Recurrence state is fp32.
  Transition: pooled features are redistributed with one 8-rank AllToAll
  so that layer 2 can run batch-parallel (core j owns batches
  {2j, 2j+1, 16+2j, 17+2j} - a mix of both batch halves, which makes every
  core's reads of the AllToAll output rank-uniform; the host unpermutes
  the final rows).
  Layer 2 (ChebConv on the 1024-node graph): batch-parallel, adjacency
  replicated. Head: h2 is transposed and all-gathered; fc1 is sharded over
  output columns (D) so reads are rank-uniform; z blocks are all-gathered
  and fc2 + log_softmax run redundantly on every core.

The FFT is folded into W1 on the host: real(FFT(x, axis=t)) = x @ Ccos and
Ccos commutes with the graph operator, so the recurrence runs on raw x
with W1_eff[k] = Ccos @ W1[k].
"""

import sys

if "/opt/trn_rl_repo" not in sys.path:
    sys.path.insert(0, "/opt/trn_rl_repo")

import numpy as np
import ml_dtypes

import concourse.bacc as bacc
import concourse.mybir as mybir
import concourse.bass_utils as _bu
from concourse.bass_utils import run_bass_kernel_spmd
from concourse.tile import TileContext
from concourse.masks import make_identity

_bu.upload_artifacts = lambda tmpdir: f"file://{tmpdir}"  # no bucket in sandbox

F32 = mybir.dt.float32
BF16 = mybir.dt.bfloat16
AX = mybir.AxisListType
ALU = mybir.AluOpType
ACT = mybir.ActivationFunctionType

B, N0, T, K = 32, 4096, 30, 25
G1, G2, D, C = 32, 64, 512, 10
N2 = N0 // 4
NCORES = 8
GCACHE = 12
NB = 4                 # layer-1 node shards
BL = B // 2            # 16 batches per layer-1 batch-half
TP = 32                # taps padded 30 -> 32
C1 = BL * TP           # 512 layer-1 channels per core
NBLK = N0 // NB        # 1024 nodes per layer-1 shard
P2BLK = N2 // NB       # 256 pooled nodes per layer-1 shard
B2 = 4                 # batches per layer-2 core
C2 = B2 * G1           # 128 layer-2 channels
DBLK = D // NCORES     # (unused) fc1 column split
FBLK = (N2 * G2) // NCORES  # 8192 fc1 contraction rows per core

G4 = [[0, 1, 2, 3], [4, 5, 6, 7]]
G8 = [list(range(NCORES))]


def _b16(a):
    return np.ascontiguousarray(a.astype(ml_dtypes.bfloat16))


def _dense_adj(edge_index, n):
    row = edge_index[0].astype(np.int64)
    col = edge_index[1].astype(np.int64)
    deg = np.zeros(n, np.float32)
    np.add.at(deg, row, 1.0)
    dis = np.where(deg > 0, 1.0 / np.sqrt(np.maximum(deg, 1.0)), 0.0).astype(np.float32)
    w = (-dis[row] * dis[col]).astype(np.float32)
    a = np.zeros((n, n), np.float32)
    np.add.at(a, (row, col), w)
    return a


def build_program(dbg=False):
    nc = bacc.Bacc("TRN2", target_bir_lowering=False, debug=False,
                   num_devices=NCORES)

    a1t_in = nc.dram_tensor("a1t", [N0, NBLK], BF16, kind="ExternalInput")
    m1t_in = nc.dram_tensor("m1t", [N0, NBLK], BF16, kind="ExternalInput")
    a2t_in = nc.dram_tensor("a2t", [N2, N2], BF16, kind="ExternalInput")
    x_nm_in = nc.dram_tensor("x_nm", [N0, C1], BF16, kind="ExternalInput")
    x_blk_in = nc.dram_tensor("x_blk", [NBLK, C1], F32, kind="ExternalInput")
    w1_in = nc.dram_tensor("w1a", [128, K * G1], BF16, kind="ExternalInput")
    w2_in = nc.dram_tensor("w2a", [128, K * 2 * G1], BF16, kind="ExternalInput")
    b1_in = nc.dram_tensor("b1v", [128, 1], F32, kind="ExternalInput")
    b2_in = nc.dram_tensor("b2v", [128, 2], F32, kind="ExternalInput")
    fc1w_in = nc.dram_tensor("fc1w", [FBLK, D], BF16, kind="ExternalInput")
    fc1b_in = nc.dram_tensor("fc1b", [B, D], F32, kind="ExternalInput")
    fc2w_in = nc.dram_tensor("fc2w", [D, C], BF16, kind="ExternalInput")
    fc2b_in = nc.dram_tensor("fc2b", [B, C], F32, kind="ExternalInput")

    out_t = nc.dram_tensor("out", [B, C], F32, kind="ExternalOutput")
    if dbg:
        h1_dbg = nc.dram_tensor("h1_dbg", [512, NBLK], F32, kind="ExternalOutput")
        l2i_dbg = nc.dram_tensor("l2i_dbg", [N2, C2], F32, kind="ExternalOutput")
        h2_dbg = nc.dram_tensor("h2_dbg", [256, N2], F32, kind="ExternalOutput")
        z_dbg = nc.dram_tensor("z_dbg", [B, D], F32, kind="ExternalOutput")

    cc1_in = [nc.dram_tensor(f"cc1i{i}", [NBLK, C1], BF16) for i in range(2)]
    cc1_out = [nc.dram_tensor(f"cc1o{i}", [N0, C1], BF16) for i in range(2)]
    ccp_in = nc.dram_tensor("ccp_in", [NCORES * P2BLK, 2 * G1], BF16)
    ccp_out = nc.dram_tensor("ccp_out", [NCORES * P2BLK, 2 * G1], BF16)
    cch_in = nc.dram_tensor("cch_in", [N2 * G2, B2], BF16)
    cch_out = nc.dram_tensor("cch_out", [N2 * G2, B2], BF16)
    ccz_in = nc.dram_tensor("ccz_in", [B, D], F32)
    ccz_out = nc.dram_tensor("ccz_out", [B, D], F32, addr_space="Shared")

    with TileContext(nc) as tc:
        with tc.tile_pool(name="const", bufs=1) as cpool:
            ident = cpool.tile([128, 128], F32)
            make_identity(nc, ident[:])

            # ======================= LAYER 1 =======================
            # Even/odd Chebyshev chains: T_{k} = 2*T_2*T_{k-2} - T_{k-4} with
            # M := 4*A^2 applied on PE and the -I part applied exactly on DVE:
            #   tx_k = M@tx_{k-2} - 2*tx_{k-2} - tx_{k-4}   (k >= 4)
            #   tx_2 = 0.5*M@tx_0 - tx_0 ;  tx_3 = M@tx_1 - 3*tx_1
            #   tx_1 = 0.5*(2A)@tx_0
            # Consecutive spmvs alternate chains, so the AllGather of chain X
            # overlaps the spmv of chain Y.
            # DRAM state rows are (p, t)-interleaved: stored row p*8+t holds
            # node t*128+p of the shard, so SBUF<->DRAM DMAs are contiguous.
            with tc.tile_pool(name="l1", bufs=1) as l1, \
                 tc.tile_pool(name="l1st", bufs=5) as l1st, \
                 tc.tile_pool(name="l1bf", bufs=1) as l1bf, \
                 tc.tile_pool(name="l1g", bufs=16) as l1g, \
                 tc.tile_pool(name="l1a", bufs=2) as l1a, \
                 tc.tile_pool(name="l1cm", bufs=1) as l1cm, \
                 tc.tile_pool(name="ps_y", bufs=1, space="PSUM") as ps_y, \
                 tc.tile_pool(name="ps_tr", bufs=2, space="PSUM") as ps_tr, \
                 tc.tile_pool(name="ps_ct", bufs=1, space="PSUM") as ps_ct:

                m1t = l1.tile([128, N0 // 128, NBLK], BF16)
                nc.sync.dma_start(m1t[:], m1t_in.ap().rearrange("(t p) n -> p t n", p=128))
                w1a = l1.tile([128, K, G1], BF16)
                nc.sync.dma_start(w1a[:], w1_in.ap().rearrange("p (k g) -> p k g", k=K))
                h1_sb = l1.tile([128, 4, NBLK], F32)
                nc.any.memset(h1_sb[:], 0.0)

                def l1_contract(src_f32, kk):
                    # src_f32: [128, 8, C1] fp32 node-major block state, term kk.
                    cm = l1cm.tile([128, 4, NBLK], BF16, tag="cm", name=f"cm{kk}")
                    for cht in range(4):
                        for ntg in range(2):
                            trt = ps_tr.tile([128, 4, 128], F32, tag="tr",
                                             name=f"tr{kk}_{cht}_{ntg}")
                            for j in range(4):
                                nt = 4 * ntg + j
                                nc.tensor.transpose(
                                    trt[:, j, :],
                                    src_f32[:, nt, 128 * cht:128 * (cht + 1)],
                                    ident[:])
                                nc.any.tensor_copy(
                                    out=cm[:, cht, 128 * nt:128 * (nt + 1)],
                                    in_=trt[:, j, :])
                    for cht in range(4):
                        for ch in range(NBLK // 512):
                            cps = ps_ct.tile([128, 512], F32, tag="ct",
                                             name=f"ct{kk}_{cht}_{ch}")
                            for bb in range(4):
                                nc.tensor.matmul(
                                    cps[32 * bb:32 * (bb + 1), :],
                                    w1a[32 * bb:32 * (bb + 1), kk, :],
                                    cm[32 * bb:32 * (bb + 1), cht, 512 * ch:512 * (ch + 1)],
                                    start=True, stop=True,
                                    tile_position=(32 * bb, 32 * bb))
                            nc.vector.tensor_tensor(
                                h1_sb[:, cht, 512 * ch:512 * (ch + 1)],
                                h1_sb[:, cht, 512 * ch:512 * (ch + 1)],
                                cps[:], ALU.add)

                NT0 = N0 // 128  # 32 gathered-node tiles

                def g_tile_ap(k, kt):
                    """[128, C1] stored-order tile kt of gathered term k."""
                    if k == 0:
                        base = x_nm_in.ap()
                    else:
                        base = cc1_out[k % 2].ap()
                    v = base.rearrange("(r p t) c -> r t p c", p=128, t=NBLK // 128)
                    return v[kt // (NBLK // 128), kt % (NBLK // 128)]

                tx_blk = {}
                tx_blk[0] = l1st.tile([128, NBLK // 128, C1], F32, tag="txs",
                                      name="txs0")
                nc.sync.dma_start(tx_blk[0][:],
                                  x_blk_in.ap().rearrange("(t p) c -> p t c", p=128))
                l1_contract(tx_blk[0], 0)

                for k in range(1, K):
                    gsrc = 0 if k <= 2 else k - 2
                    # stream gathered tiles; cache the last GCACHE for group 2
                    gk = {}
                    tx_new = l1st.tile([128, NBLK // 128, C1], F32, tag="txs",
                                       name=f"txs{k}")
                    txbf = l1bf.tile([128, NBLK // 128, C1], BF16, tag="txbf",
                                     name=f"txbf{k}")
                    for grp in range(2):
                        yp = ps_y.tile([128, 4, 512], F32, tag="y", name=f"y{k}_{grp}")
                        kts = (list(range(NT0)) if grp == 0
                               else list(range(NT0 - GCACHE, NT0))
                               + list(range(NT0 - GCACHE)))
                        for kt in kts:
                            if kt in gk:
                                gkt = gk.pop(kt)
                            else:
                                gkt = l1g.tile([128, C1], BF16, tag="gkt",
                                               name=f"g{k}_{grp}_{kt}")
                                nc.sync.dma_start(gkt[:], g_tile_ap(gsrc, kt))
                            if grp == 0 and kt >= NT0 - GCACHE:
                                gk[kt] = gkt
                            if k == 1:
                                op = l1a.tile([128, NBLK], BF16, tag="aop",
                                              name=f"a{grp}_{kt}")
                                nc.sync.dma_start(
                                    op[:], a1t_in.ap().rearrange(
                                        "(t p) n -> t p n", p=128)[kt])
                                opv = op[:, 512 * grp:512 * (grp + 1)]
                            else:
                                opv = m1t[:, kt, 512 * grp:512 * (grp + 1)]
                            for oi in range(4):
                                nc.tensor.matmul(
                                    yp[:, oi, :],
                                    opv[:, 128 * oi:128 * (oi + 1)],
                                    gkt[:],
                                    start=(kt == kts[0]), stop=(kt == kts[-1]))
                        for oi in range(4):
                            ot = 4 * grp + oi
                            yap = yp[:, oi, :]
                            o = tx_new[:, ot, :]
                            if k == 1:
                                nc.vector.tensor_scalar_mul(o, yap, 0.5)
                            elif k == 2:
                                nc.vector.tensor_scalar_mul(o, yap, 0.5)
                                nc.vector.tensor_tensor(o, o, tx_blk[0][:, ot, :],
                                                        ALU.subtract)
                            elif k == 3:
                                p1 = tx_blk[1][:, ot, :]
                                nc.vector.tensor_tensor(o, yap, p1, ALU.subtract)
                                nc.vector.tensor_tensor(o, o, p1, ALU.subtract)
                                nc.vector.tensor_tensor(o, o, p1, ALU.subtract)
                            else:
                                p2 = tx_blk[k - 2][:, ot, :]
                                nc.vector.tensor_tensor(o, yap, p2, ALU.subtract)
                                nc.vector.tensor_tensor(o, o, p2, ALU.subtract)
                                nc.vector.tensor_tensor(o, o, tx_blk[k - 4][:, ot, :],
                                                        ALU.subtract)
                            nc.vector.tensor_copy(txbf[:, ot, :], o)
                    tx_blk[k] = tx_new
                    # store in (p, t)-interleaved order, then 4-rank AllGather
                    cin, cout = cc1_in[k % 2], cc1_out[k % 2]
                    nc.sync.dma_start(
                        cin.ap().rearrange("(p t) c -> p t c", t=NBLK // 128),
                        txbf[:])
                    nc.gpsimd.collective_compute(
                        "AllGather", ALU.bypass, replica_groups=G4,
                        ins=[cin.ap()], outs=[cout.ap()])
                    l1_contract(tx_new, k)
                    tx_blk.pop(k - 4, None)

                # bias + relu + maxpool4 along nodes
                b1v = l1.tile([128, 1], F32)
                nc.sync.dma_start(b1v[:], b1_in.ap())
                h1p = l1.tile([128, 4, P2BLK], F32)
                for cht in range(4):
                    nc.scalar.activation(h1_sb[:, cht, :], h1_sb[:, cht, :], ACT.Relu,
                                         bias=b1v[:])
                    h4 = h1_sb[:, cht, :].rearrange("p (n f) -> p n f", f=4)
                    nc.vector.tensor_tensor(h1p[:, cht, :], h4[:, :, 0], h4[:, :, 1],
                                            ALU.max)
                    nc.vector.tensor_tensor(h1p[:, cht, :], h1p[:, cht, :], h4[:, :, 2],
                                            ALU.max)
                    nc.vector.tensor_tensor(h1p[:, cht, :], h1p[:, cht, :], h4[:, :, 3],
                                            ALU.max)
                if dbg:
                    nc.sync.dma_start(
                        h1_dbg.ap().rearrange("(t p) n -> p t n", p=128), h1_sb[:])

                # transpose pooled block -> [n2_local, (b_loc, g)] bf16
                h1pt = l1.tile([128, P2BLK // 128, BL * G1], BF16)
                for cht in range(4):
                    for nt in range(P2BLK // 128):
                        trp = ps_tr.tile([128, 128], F32, tag="tr")
                        nc.tensor.transpose(
                            trp[:], h1p[:, cht, 128 * nt:128 * (nt + 1)], ident[:])
                        nc.any.tensor_copy(
                            out=h1pt[:, nt, 128 * cht:128 * (cht + 1)], in_=trp[:])

                ccp_iv = ccp_in.ap().rearrange("(s t p) c -> s p t c", p=128,
                                               t=P2BLK // 128)
                for s in range(NCORES):
                    nc.sync.dma_start(ccp_iv[s],
                                      h1pt[:, :, 64 * s:64 * (s + 1)])
                nc.gpsimd.collective_compute(
                    "AllToAll", ALU.bypass, replica_groups=G8,
                    ins=[ccp_in.ap()], outs=[ccp_out.ap()])

            # ======================= LAYER 2 =======================
            # ccp_out rows: src_rank * P2BLK + n2l, src_rank = bh*4 + nb;
            # cols: (b_pair 2, g 32). My batches (c2 order): b = bh*2 + pair.
            with tc.tile_pool(name="l2", bufs=1) as l2, \
                 tc.tile_pool(name="l2st", bufs=3) as l2st, \
                 tc.tile_pool(name="l2bf", bufs=2) as l2bf, \
                 tc.tile_pool(name="l2cm", bufs=2) as l2cm, \
                 tc.tile_pool(name="ps2_y", bufs=2, space="PSUM") as ps2_y, \
                 tc.tile_pool(name="ps2_tr", bufs=2, space="PSUM") as ps2_tr, \
                 tc.tile_pool(name="ps2_ct", bufs=2, space="PSUM") as ps2_ct:

                a2t = l2.tile([128, N2 // 128, N2], BF16)
                nc.sync.dma_start(a2t[:], a2t_in.ap().rearrange("(t p) n -> p t n", p=128))
                w2a = l2.tile([128, K, 2, G1], BF16)
                nc.sync.dma_start(
                    w2a[:], w2_in.ap().rearrange("p (k h g) -> p k h g", k=K, h=2))

                # init state: [128 n2, 8 nt, (b 4, g 32)] from ccp_out
                st0_bf = l2bf.tile([128, N2 // 128, C2], BF16, tag="st2bf")
                ccp_v = ccp_out.ap().rearrange(
                    "(bh nb t p) c -> bh nb p t c", bh=2, nb=NB, t=P2BLK // 128)
                for bh in range(2):
                    for nb in range(NB):
                        # dest cols [bh*64, +64) = (b = bh*2 + pair, g)
                        nc.sync.dma_start(
                            st0_bf[:, 2 * nb:2 * (nb + 1),
                                   64 * bh:64 * (bh + 1)],
                            ccp_v[bh, nb])
                st0 = l2st.tile([128, N2 // 128, C2], F32, tag="st2")
                nc.vector.tensor_copy(st0[:], st0_bf[:])
                if dbg:
                    nc.sync.dma_start(
                        l2i_dbg.ap().rearrange("(t p) c -> p t c", p=128), st0[:])

                h2a = l2.tile([128, 2, N2], F32)
                nc.any.memset(h2a[:], 0.0)

                def l2_contract(src_f32, kk):
                    cm = l2cm.tile([128, N2], BF16, tag="cm2")
                    for nt in range(N2 // 128):
                        trp = ps2_tr.tile([128, 128], F32, tag="tr2")
                        nc.tensor.transpose(trp[:], src_f32[:, nt, :], ident[:])
                        nc.any.tensor_copy(
                            out=cm[:, 128 * nt:128 * (nt + 1)], in_=trp[:])
                    for hh in range(2):
                        cps = ps2_ct.tile([128, N2], F32, tag="ct2")
                        for ch in range(N2 // 512):
                            for bb in range(4):
                                nc.tensor.matmul(
                                    cps[32 * bb:32 * (bb + 1), 512 * ch:512 * (ch + 1)],
                                    w2a[32 * bb:32 * (bb + 1), kk, hh, :],
                                    cm[32 * bb:32 * (bb + 1), 512 * ch:512 * (ch + 1)],
                                    start=True, stop=True,
                                    tile_position=(32 * bb, 32 * bb))
                        nc.vector.tensor_tensor(h2a[:, hh, :], h2a[:, hh, :],
                                                cps[:], ALU.add)

                l2_contract(st0, 0)
                tx2_pp = None
                tx2_prev = st0
                gath2 = st0_bf
                for k in range(1, K):
                    yps = []
                    for g in range(2):
                        yp = ps2_y.tile([128, 4, 128], F32, tag="y2")
                        yps.append(yp)
                        for oi in range(4):
                            ot = 4 * g + oi
                            for kt in range(N2 // 128):
                                nc.tensor.matmul(
                                    yp[:, oi, :],
                                    a2t[:, kt, 128 * ot:128 * (ot + 1)],
                                    gath2[:, kt, :],
                                    start=(kt == 0), stop=(kt == N2 // 128 - 1))
                    tx2_new = l2st.tile([128, N2 // 128, C2], F32, tag="st2")
                    g2bf = l2bf.tile([128, N2 // 128, C2], BF16, tag="st2bf")
                    for ot in range(8):
                        yap = yps[ot // 4][:, ot % 4, :]
                        if k == 1:
                            nc.vector.tensor_scalar_mul(tx2_new[:, ot, :], yap, 0.5)
                        else:
                            nc.vector.tensor_tensor(tx2_new[:, ot, :], yap,
                                                    tx2_pp[:, ot, :], ALU.subtract)
                        nc.vector.tensor_copy(g2bf[:, ot, :], tx2_new[:, ot, :])
                    l2_contract(tx2_new, k)
                    gath2 = g2bf
                    tx2_pp = tx2_prev
                    tx2_prev = tx2_new

                # bias + relu, then transpose h2 -> [n2, (b, g2)] bf16
                b2v = l2.tile([128, 2], F32)
                nc.sync.dma_start(b2v[:], b2_in.ap())
                h2r = l2.tile([128, 2, N2], F32)
                for hh in range(2):
                    nc.scalar.activation(h2r[:, hh, :], h2a[:, hh, :], ACT.Relu,
                                         bias=b2v[:, hh:hh + 1])
                if dbg:
                    nc.sync.dma_start(
                        h2_dbg.ap().rearrange("(t p) n -> p t n", p=128), h2r[:])
                # build f-major features: ft_sb[n2_l, nt, (g2 64, b 4)]
                ft_sb = l2.tile([128, N2 // 128, G2 * B2], BF16)
                for hh in range(2):
                    for nt in range(N2 // 128):
                        trp = ps2_tr.tile([128, 128], F32, tag="tr2")
                        nc.tensor.transpose(trp[:], h2r[:, hh, 128 * nt:128 * (nt + 1)],
                                            ident[:])
                        # cols of trp: (b 4, g2r 32) -> dest (g2 = hh*32+g2r, b)
                        nc.any.tensor_copy(
                            out=ft_sb[:, nt, :].rearrange("p (g b) -> p g b", g=G2)[
                                :, 32 * hh:32 * (hh + 1), :],
                            in_=trp[:].rearrange("p (b g) -> p g b", b=4))
                # AllToAll: slot j = my rows f in [FBLK*j, FBLK*(j+1))
                # cch_in rows (j, n2_l 128, g2 64), cols b
                nc.sync.dma_start(
                    cch_in.ap().rearrange("(j nl g) b -> nl j (g b)",
                                          j=NCORES, nl=128),
                    ft_sb[:])
                nc.gpsimd.collective_compute(
                    "AllToAll", ALU.bypass, replica_groups=G8,
                    ins=[cch_in.ap()], outs=[cch_out.ap()])

            # ======================= HEAD =======================
            with tc.tile_pool(name="fc", bufs=1) as fc, \
                 tc.tile_pool(name="fcw", bufs=4) as fcw, \
                 tc.tile_pool(name="ps3", bufs=2, space="PSUM") as ps3, \
                 tc.tile_pool(name="ps3z", bufs=1, space="PSUM") as ps3z:

                # flatT: my f-block x all batches: [128 p, 64 kt, 32 (r 8, b 4)]
                flt = fc.tile([128, FBLK // 128, B], BF16, tag="flt")
                zps = ps3z.tile([32, D], F32)
                cch_v = cch_out.ap().rearrange(
                    "(r kt p) b -> r p kt b", r=NCORES, kt=FBLK // 128)
                for r in range(NCORES):
                    nc.sync.dma_start(flt[:, :, B2 * r:B2 * (r + 1)], cch_v[r])
                for kt in range(FBLK // 128):
                    fw = fcw.tile([128, D], BF16, tag="fw")
                    nc.sync.dma_start(
                        fw[:], fc1w_in.ap().rearrange("(kt p) d -> kt p d", p=128)[kt])
                    nc.tensor.matmul(zps[:], flt[:, kt, :], fw[:],
                                     start=(kt == 0), stop=(kt == FBLK // 128 - 1))
                zblk = fc.tile([32, D], F32)
                nc.vector.tensor_copy(zblk[:], zps[:])
                nc.sync.dma_start(ccz_in.ap(), zblk[:])
                nc.gpsimd.collective_compute(
                    "AllReduce", ALU.add, replica_groups=G8,
                    ins=[ccz_in.ap()], outs=[ccz_out.ap()])
                zfull = fc.tile([32, D], F32)
                nc.sync.dma_start(zfull[:], ccz_out.ap())
                zb = fc.tile([32, D], F32)
                nc.sync.dma_start(zb[:], fc1b_in.ap())
                nc.vector.tensor_tensor(zfull[:], zfull[:], zb[:], ALU.add)
                zr = fc.tile([32, D], F32)
                nc.scalar.activation(zr[:], zfull[:], ACT.Relu)
                if dbg:
                    nc.sync.dma_start(z_dbg.ap(), zr[:])

                # fc2: transpose z, then [32, 10] = sum_kt zT[kt].T @ fc2w[kt]
                f2w = fc.tile([128, 4, C], BF16)
                nc.sync.dma_start(f2w[:],
                                  fc2w_in.ap().rearrange("(t p) c -> p t c", p=128))
                lps = ps3.tile([32, C], F32, tag="lg")
                for t4 in range(4):
                    ztp = ps3.tile([128, 32], F32, tag="zt")
                    nc.tensor.transpose(ztp[:], zr[:, 128 * t4:128 * (t4 + 1)],
                                        ident[:32, :32])
                    zts = fc.tile([128, 32], BF16, tag="zts")
                    nc.any.tensor_copy(out=zts[:], in_=ztp[:])
                    nc.tensor.matmul(lps[:], zts[:], f2w[:, t4, :],
                                     start=(t4 == 0), stop=(t4 == 3))
                logits = fc.tile([32, C], F32)
                f2b = fc.tile([32, C], F32)
                nc.sync.dma_start(f2b[:], fc2b_in.ap())
                nc.vector.tensor_tensor(logits[:], lps[:], f2b[:], ALU.add)

                mx = fc.tile([32, 1], F32)
                nc.vector.tensor_reduce(mx[:], logits[:], axis=AX.X, op=ALU.max)
                sh = fc.tile([32, C], F32)
                nc.vector.tensor_tensor(sh[:], logits[:], mx[:].to_broadcast((32, C)),
                                        ALU.subtract)
                ex = fc.tile([32, C], F32)
                nc.scalar.activation(ex[:], sh[:], ACT.Exp)
                sm = fc.tile([32, 1], F32)
                nc.vector.tensor_reduce(sm[:], ex[:], axis=AX.X, op=ALU.add)
                lg = fc.tile([32, 1], F32)
                nc.scalar.activation(lg[:], sm[:], ACT.Ln)
                res = fc.tile([32, C], F32)
                nc.vector.tensor_tensor(res[:], sh[:], lg[:].to_broadcast((32, C)),
                                        ALU.subtract)
                nc.sync.dma_start(out_t.ap(), res[:])

    nc.compile()
    return nc


def _identity_cos():
    t = np.arange(T)
    f = np.arange(T)
    return np.cos(2.0 * np.pi * np.outer(t, f) / T).astype(np.float32)


def make_inputs(x, edge_index0, edge_index2, W1, b1, W2, b2,
                fc1_w, fc1_b, fc2_w, fc2_b):
    """Build the 8 per-core input maps."""
    A0 = _dense_adj(np.asarray(edge_index0), N0)
    A2 = _dense_adj(np.asarray(edge_index2), N2)
    A1T2 = _b16((2.0 * A0).T)              # [N0, N0] cols -> row blocks
    M1T = _b16((4.0 * (A0 @ A0)).T)        # 4*A^2, transposed
    A2T2 = _b16((2.0 * A2).T)              # [N2, N2]
    # (p, t)-interleaved storage order for gathered layer-1 state rows
    il = np.arange(N0)
    rank_, rem = il // NBLK, il % NBLK
    p_, t_ = rem // (NBLK // 128), rem % (NBLK // 128)
    node_of_row = rank_ * NBLK + t_ * 128 + p_

    Ccos = _identity_cos()
    W1e = np.einsum("tf,kfg->ktg", Ccos, np.asarray(W1, np.float32))  # [K, 30, G1]
    w1a = np.zeros((128, K, G1), np.float32)
    for bb in range(4):
        w1a[32 * bb:32 * bb + 30] = W1e.transpose(1, 0, 2)
    w1a = _b16(w1a.reshape(128, K * G1))

    W2f = np.asarray(W2, np.float32)       # [K, G1, G2]
    w2a = np.zeros((128, K, 2, G1), np.float32)
    for bb in range(4):
        for hh in range(2):
            w2a[32 * bb:32 * bb + 32, :, hh, :] = \
                W2f[:, :, 32 * hh:32 * hh + 32].transpose(1, 0, 2)
    w2a = _b16(w2a.reshape(128, K * 2 * G1))

    b1v = np.tile(np.asarray(b1, np.float32), 4).reshape(128, 1)
    b2f = np.asarray(b2, np.float32)
    b2v = np.stack([np.tile(b2f[:32], 4), np.tile(b2f[32:], 4)], 1).astype(np.float32)

    fc1b = np.tile(np.asarray(fc1_b, np.float32)[None, :], (B, 1))
    fc2b = np.tile(np.asarray(fc2_b, np.float32)[None, :], (B, 1))
    fc2w = _b16(np.asarray(fc2_w, np.float32))

    xf = np.asarray(x, np.float32)          # [B, N0, T]
    fc1wf = np.asarray(fc1_w, np.float32)   # [N2*G2, D]

    ins = []
    for core in range(NCORES):
        bh, nb = core // 4, core % 4
        # layer-1 channels: c = b_loc*32 + t, batches 16*bh + b_loc
        xs = xf[16 * bh:16 * (bh + 1)]          # [16, N0, 30]
        x_nm = np.zeros((N0, C1), np.float32)
        x_nm.reshape(N0, BL, TP)[:, :, :T] = xs.transpose(1, 0, 2)
        x_blk = x_nm[NBLK * nb:NBLK * (nb + 1)].copy()
        x_nm = x_nm[node_of_row]
        ins.append({
            "a1t": np.ascontiguousarray(A1T2[:, NBLK * nb:NBLK * (nb + 1)]),
            "m1t": np.ascontiguousarray(M1T[:, NBLK * nb:NBLK * (nb + 1)]),
            "a2t": A2T2,
            "x_nm": _b16(x_nm),
            "x_blk": x_blk,
            "w1a": w1a, "w2a": w2a, "b1v": b1v, "b2v": b2v,
            "fc1w": _b16(fc1wf[FBLK * core:FBLK * (core + 1), :]),
            "fc1b": fc1b, "fc2b": fc2b, "fc2w": fc2w,
        })
    return ins


def batch_perm():
    """flat row order (r, b_c2) -> global batch id."""
    perm = []
    for r in range(NCORES):
        for b_c2 in range(4):
            bh, pair = b_c2 // 2, b_c2 % 2
            perm.append(16 * bh + 2 * r + pair)
    return np.array(perm)


_CACHED = {}


def kernel(**inputs):
    if "nc" not in _CACHED:
        _CACHED["nc"] = build_program(dbg=False)
    nc = _CACHED["nc"]
    ins = make_inputs(**inputs)
    res = run_bass_kernel_spmd(nc, ins, core_ids=list(range(NCORES)))
    out = np.zeros((B, C), np.float32)
    out[batch_perm()] = res.results[0]["out"]
    return out



# revision 10
# speedup vs baseline: 2.4708x; 2.4708x over previous
"""NetTGCN forward pass on 8 Trainium2 NeuronCores (Bass/Tile).

Key algorithmic move: the reference's real(FFT) along the 30 time taps is a
rank-16 linear map (cos(2*pi*t*f/30) has identical columns for f and 30-f),
so layer 1's Chebyshev recurrence runs on 16 frequency channels per batch
instead of 30 taps - half the spmv FLOPs of a direct fold.

Sharding:
  Layer 1 (4096-node graph): 2-way node-shard x 4-way batch-shard. Per core:
  8 batches x 16 freqs = 128 channels, 2048 own nodes. The state is kept
  CHANNEL-major [128 c, 2048 n]; the spmv is out = state_blk.T @ M^T-rows
  (stationary = node-major state blocks from the gathered DRAM copy, moving =
  SBUF-resident M^T shard, N=512), which directly produces the channel-major
  next state, so the per-k W-contraction needs no transposes. The per-step
  exchange is a 2-rank AllGather (pairs (c, c+4)) of the XBAR-DMA-transposed
  fp16 state (0.5 MB wire, ~16 us), hidden under the other Chebyshev chain's
  spmv (even/odd chains via M = 4*A'^2). fp16 everywhere in layer 1 (states
  included): simulated end-to-end error 1.8e-3.
  Core (h, q) = core h*4+q owns node half h and batches b_loc -> global
  batch 4*b_loc + q; L2 core j owns batches 4j..4j+3.
  Layer 2 (1024-node graph): batch-parallel (core j handles batches
  4j..4j+3 after an 8-rank AllToAll), zero collectives in the loop,
  same channel-major spmv structure, A2 resident, fc1w prefetched meanwhile.
  Head: h2 features redistributed with an 8-rank AllToAll so fc1 is sharded
  over its 65536-row contraction; partial z AllReduced; fc2 + log_softmax
  computed redundantly on every core. Host un-permutes the 32 rows.
"""

import sys

if "/opt/trn_rl_repo" not in sys.path:
    sys.path.insert(0, "/opt/trn_rl_repo")

import numpy as np

import concourse.bacc as bacc
import concourse.mybir as mybir
import concourse.bass_utils as _bu
from concourse.bass_utils import run_bass_kernel_spmd
from concourse.tile import TileContext
from concourse.masks import make_identity

_bu.upload_artifacts = lambda tmpdir: f"file://{tmpdir}"  # no bucket in sandbox

F16 = mybir.dt.float16
F32 = mybir.dt.float32
AX = mybir.AxisListType
ALU = mybir.AluOpType
ACT = mybir.ActivationFunctionType

B, N0, T, K = 32, 4096, 30, 25
G1, G2, D, C = 32, 64, 512, 10
N2 = N0 // 4
NF = 16                 # rank of the real-FFT cosine map
NCORES = 8
NH = N0 // 2            # 2048 own nodes per core (node half)
P2H = N2 // 2           # 512 own pooled nodes
FBLK = (N2 * G2) // NCORES  # 8192 fc1 contraction rows per core

GPAIR = [[0, 4], [1, 5], [2, 6], [3, 7]]
G8 = [list(range(NCORES))]


def _f16(a):
    return np.ascontiguousarray(np.asarray(a, np.float32).astype(np.float16))


def _dense_adj(edge_index, n):
    row = edge_index[0].astype(np.int64)
    col = edge_index[1].astype(np.int64)
    deg = np.zeros(n, np.float32)
    np.add.at(deg, row, 1.0)
    dis = np.where(deg > 0, 1.0 / np.sqrt(np.maximum(deg, 1.0)), 0.0).astype(np.float32)
    w = (-dis[row] * dis[col]).astype(np.float32)
    a = np.zeros((n, n), np.float32)
    np.add.at(a, (row, col), w)
    return a


def build_program(dbg=False):
    nc = bacc.Bacc("TRN2", target_bir_lowering=False, debug=False,
                   num_devices=NCORES)

    x_cm_in = nc.dram_tensor("x_cm", [128, 2 * NH], F16, kind="ExternalInput")
    c16_in = nc.dram_tensor("c16w", [128, 2 * 128], F16, kind="ExternalInput")
    m1t_in = nc.dram_tensor("m1t", [N0, NH], F16, kind="ExternalInput")
    a1t_in = nc.dram_tensor("a1t", [N0, NH], F16, kind="ExternalInput")
    a2t_in = nc.dram_tensor("a2t", [N2, N2], F16, kind="ExternalInput")
    w1_in = nc.dram_tensor("w1a", [128, K * 2 * 128], F16, kind="ExternalInput")
    w2_in = nc.dram_tensor("w2a", [128, K * 2 * 128], F16, kind="ExternalInput")
    b1_in = nc.dram_tensor("b1v", [128, 1], F32, kind="ExternalInput")
    b2_in = nc.dram_tensor("b2v", [128, 1], F32, kind="ExternalInput")
    fc1w_in = nc.dram_tensor("fc1w", [FBLK, D], F16, kind="ExternalInput")
    fc1b_in = nc.dram_tensor("fc1b", [B, D], F32, kind="ExternalInput")
    fc2w_in = nc.dram_tensor("fc2w", [D, C], F16, kind="ExternalInput")
    fc2b_in = nc.dram_tensor("fc2b", [B, C], F32, kind="ExternalInput")

    out_t = nc.dram_tensor("out", [B, C], F32, kind="ExternalOutput")
    if dbg:
        h1_dbg = nc.dram_tensor("h1_dbg", [256, NH], F32, kind="ExternalOutput")
        l2i_dbg = nc.dram_tensor("l2i_dbg", [128, N2], F32, kind="ExternalOutput")
        h2_dbg = nc.dram_tensor("h2_dbg", [256, N2], F32, kind="ExternalOutput")
        z_dbg = nc.dram_tensor("z_dbg", [B, D], F32, kind="ExternalOutput")

    cc1_in = [nc.dram_tensor(f"cc1i{i}", [NH, 128], F16) for i in range(2)]
    cc1_out = [nc.dram_tensor(f"cc1o{i}", [N0, 128], F16) for i in range(2)]
    ccp_in = nc.dram_tensor("ccp_in", [256, P2H], F16)
    ccp_out = nc.dram_tensor("ccp_out", [256, P2H], F16)
    cch_in = nc.dram_tensor("cch_in", [NCORES * 4, FBLK], F16)
    cch_out = nc.dram_tensor("cch_out", [NCORES * 4, FBLK], F16)
    ccz_in = nc.dram_tensor("ccz_in", [B, D], F32)
    ccz_out = nc.dram_tensor("ccz_out", [B, D], F32, addr_space="Shared")

    with TileContext(nc) as tc:
        # ======================= LAYER 1 =======================
        with tc.tile_pool(name="l1c", bufs=1) as l1c, \
             tc.tile_pool(name="l1mv", bufs=1) as l1mv, \
             tc.tile_pool(name="l1st", bufs=5) as l1st, \
             tc.tile_pool(name="l1g", bufs=3) as l1g, \
             tc.tile_pool(name="l1nm", bufs=2) as l1nm, \
             tc.tile_pool(name="ps_y", bufs=3, space="PSUM") as ps_y, \
             tc.tile_pool(name="ps_ct", bufs=2, space="PSUM") as ps_ct:

            w1a = l1c.tile([128, K, 2, 128], F16)
            nc.sync.dma_start(
                w1a[:], w1_in.ap().rearrange("p (k b c) -> p k b c", k=K, b=2))
            b1v = l1c.tile([128, 1], F32)
            nc.sync.dma_start(b1v[:], b1_in.ap())
            h1_sb = l1c.tile([128, 2, NH], F32)
            nc.any.memset(h1_sb[:], 0.0)

            # moving-operand buffer: holds a1t for k=1, then m1t for k>=2.
            # Bulk loads ride the scalar-engine HWDGE queue so the
            # latency-critical sync-queue DMAs are not stuck behind them.
            mv = l1mv.tile([128, 32, NH], F16)
            a1_v = a1t_in.ap().rearrange("(t p) n -> t p n", p=128)
            m1_v = m1t_in.ap().rearrange("(t p) n -> t p n", p=128)
            for mb in range(32):
                nc.scalar.dma_start(mv[:, mb, :], a1_v[mb])

            tx = {}

            def xbar_ag(k):
                nm = l1nm.tile([128, NH // 128, 128], F16, tag="nm",
                               name=f"nm{k}")
                nc.sync.dma_start_transpose(nm[:], tx[k][:])
                cin, cout = cc1_in[k % 2], cc1_out[k % 2]
                nc.sync.dma_start(
                    cin.ap().rearrange("(t p) c -> p t c", p=128), nm[:])
                nc.gpsimd.collective_compute(
                    "AllGather", ALU.bypass, replica_groups=GPAIR,
                    ins=[cin.ap()], outs=[cout.ap()])

            def contract(k):
                for bb in range(2):
                    for ns in range(4):
                        cps = ps_ct.tile([128, 512], F32, tag="ct",
                                         name=f"ct{k}_{bb}_{ns}")
                        nc.tensor.matmul(cps[:], w1a[:, k, bb, :],
                                         tx[k][:, 512 * ns:512 * (ns + 1)],
                                         start=True, stop=True)
                        nc.vector.tensor_tensor(
                            h1_sb[:, bb, 512 * ns:512 * (ns + 1)],
                            h1_sb[:, bb, 512 * ns:512 * (ns + 1)],
                            cps[:], ALU.add)

            # ---- x' = x @ C16 (channel-major) ----
            with tc.tile_pool(name="l1x", bufs=1) as l1x:
                c16 = l1x.tile([128, 2, 128], F16)
                nc.sync.dma_start(
                    c16[:], c16_in.ap().rearrange("p (b c) -> p b c", b=2))
                x_v = x_cm_in.ap().rearrange("p (b n) -> p b n", b=2)
                tx[0] = l1st.tile([128, NH], F16, tag="tx", name="tx0")
                for bb2 in range(2):
                    xh = l1x.tile([128, NH], F16, tag="xh", name=f"xh{bb2}")
                    nc.sync.dma_start(xh[:], x_v[:, bb2, :])
                    for ns in range(4):
                        xps = ps_ct.tile([128, 512], F32, tag="ct",
                                         name=f"xp{bb2}_{ns}")
                        nc.tensor.matmul(xps[:], c16[:, bb2, :],
                                         xh[:, 512 * ns:512 * (ns + 1)],
                                         start=True, stop=True)
                        o = tx[0][:, 512 * ns:512 * (ns + 1)]
                        if bb2 == 0:
                            nc.vector.tensor_copy(o, xps[:])
                        else:
                            nc.vector.tensor_tensor(o, o, xps[:], ALU.add)
                xbar_ag(0)

            # ---- Chebyshev steps; contract(k-1) emitted between spmvs ----
            for k in range(1, K):
                gi = 0 if k == 1 else k % 2   # k=1 consumes the x' gather
                gsrc = cc1_out[gi].ap().rearrange("(t p) c -> p t c", p=128)
                tx[k] = l1st.tile([128, NH], F16, tag="tx", name=f"tx{k}")
                stt = []
                for hb in range(2):
                    s = l1g.tile([128, 16, 128], F16, tag="g",
                                 name=f"g{k}_{hb}")
                    nc.sync.dma_start(s[:], gsrc[:, 16 * hb:16 * (hb + 1), :])
                    stt.append(s)
                for half in range(2):
                    yp = ps_y.tile([128, 2, 512], F32, tag="y",
                                   name=f"y{k}_{half}")
                    for mb in range(32):
                        for j in range(2):
                            nc.tensor.matmul(
                                yp[:, j, :], stt[mb // 16][:, mb % 16, :],
                                mv[:, mb, 1024 * half + 512 * j:
                                   1024 * half + 512 * (j + 1)],
                                start=(mb == 0), stop=(mb == 31))
                    o = tx[k][:, 1024 * half:1024 * (half + 1)]
                    ypf = yp[:].rearrange("p a b -> p (a b)")
                    if k == 1:
                        nc.vector.tensor_scalar_mul(o, ypf, 0.5)
                    elif k == 2:
                        nc.vector.tensor_scalar_mul(o, ypf, 0.5)
                        nc.vector.tensor_tensor(
                            o, o, tx[0][:, 1024 * half:1024 * (half + 1)],
                            ALU.subtract)
                    elif k == 3:
                        p1 = tx[1][:, 1024 * half:1024 * (half + 1)]
                        nc.vector.tensor_tensor(o, ypf, p1, ALU.subtract)
                        nc.vector.tensor_tensor(o, o, p1, ALU.subtract)
                        nc.vector.tensor_tensor(o, o, p1, ALU.subtract)
                    else:
                        p2 = tx[k - 2][:, 1024 * half:1024 * (half + 1)]
                        p4 = tx[k - 4][:, 1024 * half:1024 * (half + 1)]
                        nc.vector.tensor_tensor(o, ypf, p2, ALU.subtract)
                        nc.vector.tensor_tensor(o, o, p2, ALU.subtract)
                        nc.vector.tensor_tensor(o, o, p4, ALU.subtract)
                if k == 1:
                    # refill the moving buffer with m1t now that a1t is done
                    for mb in range(32):
                        nc.scalar.dma_start(mv[:, mb, :], m1_v[mb])
                if k < K - 2:
                    xbar_ag(k)
                contract(k - 1)
                tx.pop(k - 4, None)
            contract(K - 1)

            # ---- bias + relu + maxpool4 along nodes ----
            h1p = l1c.tile([128, 2, P2H], F16)
            for bb in range(2):
                nc.scalar.activation(h1_sb[:, bb, :], h1_sb[:, bb, :],
                                     ACT.Relu, bias=b1v[:])
                h4 = h1_sb[:, bb, :].rearrange("p (n f) -> p n f", f=4)
                nc.vector.tensor_tensor(h1p[:, bb, :], h4[:, :, 0],
                                        h4[:, :, 1], ALU.max)
                nc.vector.tensor_tensor(h1p[:, bb, :], h1p[:, bb, :],
                                        h4[:, :, 2], ALU.max)
                nc.vector.tensor_tensor(h1p[:, bb, :], h1p[:, bb, :],
                                        h4[:, :, 3], ALU.max)
            if dbg:
                nc.sync.dma_start(
                    h1_dbg.ap().rearrange("(b p) n -> p b n", p=128), h1_sb[:])

            # 8-rank AllToAll of pooled features. Batch ownership is chosen so
            # slot j (rows 32j..32j+32 = b_loc j's g-rows x own 512 nodes) is
            # exactly what L2 core j needs from this core; the output blocks
            # are then read rank-uniformly.
            nc.sync.dma_start(
                ccp_in.ap().rearrange("(b p) c -> p b c", p=128), h1p[:])
            nc.gpsimd.collective_compute(
                "AllToAll", ALU.bypass, replica_groups=G8,
                ins=[ccp_in.ap()], outs=[ccp_out.ap()])

        # ======================= LAYER 2 =======================
        with tc.tile_pool(name="l2c", bufs=1) as l2c, \
             tc.tile_pool(name="l2st", bufs=3) as l2st, \
             tc.tile_pool(name="l2nm", bufs=2) as l2nm:

            a2t = l2c.tile([128, N2 // 128, N2], F16)
            nc.scalar.dma_start(
                a2t[:], a2t_in.ap().rearrange("(t p) n -> p t n", p=128))
            w2a = l2c.tile([128, K, 2, 128], F16)
            nc.scalar.dma_start(
                w2a[:], w2_in.ap().rearrange("p (k h c) -> p k h c", k=K, h=2))
            b2v = l2c.tile([128, 1], F32)
            nc.sync.dma_start(b2v[:], b2_in.ap())
            # preload fc1w for the head while layer 2 computes
            fc1w = l2c.tile([128, FBLK // 128, D], F16)
            nc.scalar.dma_start(
                fc1w[:], fc1w_in.ap().rearrange("(t p) d -> p t d", p=128))
            h2_sb = l2c.tile([128, 2, N2], F32)
            nc.any.memset(h2_sb[:], 0.0)

            with tc.tile_pool(name="ps2_y", bufs=2, space="PSUM") as ps2_y, \
                 tc.tile_pool(name="ps2_ct", bufs=2, space="PSUM") as ps2_ct:

                tx2 = {}
                nm2 = {}
                # out block r=(h', q') = core r's slot for me: batch 4*my_j+q'
                # (g1-rows) x n2-half h'
                tx2[0] = l2st.tile([128, N2], F16, tag="tx2", name="tx20")
                for hp in range(2):
                    for qp in range(4):
                        nc.sync.dma_start(
                            tx2[0][32 * qp:32 * (qp + 1),
                                   512 * hp:512 * (hp + 1)],
                            ccp_out.ap()[32 * (4 * hp + qp):
                                         32 * (4 * hp + qp + 1), :])
                if dbg:
                    l2i = l2c.tile([128, N2], F32)
                    nc.vector.tensor_copy(l2i[:], tx2[0][:])
                    nc.sync.dma_start(l2i_dbg.ap(), l2i[:])

                def xbar2(k):
                    nm2[k] = l2nm.tile([128, N2 // 128, 128], F16, tag="nm2",
                                       name=f"nm2_{k}")
                    nc.sync.dma_start_transpose(nm2[k][:], tx2[k][:])

                def contract2(k):
                    for hh in range(2):
                        for ns in range(2):
                            cps = ps2_ct.tile([128, 512], F32, tag="ct2",
                                              name=f"c2_{k}_{hh}_{ns}")
                            nc.tensor.matmul(
                                cps[:], w2a[:, k, hh, :],
                                tx2[k][:, 512 * ns:512 * (ns + 1)],
                                start=True, stop=True)
                            nc.vector.tensor_tensor(
                                h2_sb[:, hh, 512 * ns:512 * (ns + 1)],
                                h2_sb[:, hh, 512 * ns:512 * (ns + 1)],
                                cps[:], ALU.add)

                xbar2(0)
                for k in range(1, K):
                    tx2[k] = l2st.tile([128, N2], F16, tag="tx2",
                                       name=f"tx2{k}")
                    yp = ps2_y.tile([128, 2, 512], F32, tag="y2",
                                    name=f"y2_{k}")
                    for mb in range(N2 // 128):
                        for j in range(2):
                            nc.tensor.matmul(
                                yp[:, j, :], nm2[k - 1][:, mb, :],
                                a2t[:, mb, 512 * j:512 * (j + 1)],
                                start=(mb == 0), stop=(mb == N2 // 128 - 1))
                    ypf = yp[:].rearrange("p a b -> p (a b)")
                    if k == 1:
                        nc.vector.tensor_scalar_mul(tx2[1][:], ypf, 0.5)
                    else:
                        nc.vector.tensor_tensor(tx2[k][:], ypf,
                                                tx2[k - 2][:], ALU.subtract)
                    if k < K - 1:
                        xbar2(k)
                    contract2(k - 1)
                    nm2.pop(k - 2, None)
                    tx2.pop(k - 3, None)
                contract2(K - 1)

                # bias + relu -> fp16 channel-major h2
                h2r = l2c.tile([128, 2, N2], F16)
                for hh in range(2):
                    nc.scalar.activation(h2r[:, hh, :], h2_sb[:, hh, :],
                                         ACT.Relu, bias=b2v[:])
                if dbg:
                    h2f = l2c.tile([128, 2, N2], F32)
                    nc.vector.tensor_copy(h2f[:], h2r[:])
                    nc.sync.dma_start(
                        h2_dbg.ap().rearrange("(h p) n -> p h n", p=128),
                        h2f[:])

            # =================== HEAD ===================
            with tc.tile_pool(name="hd", bufs=1) as hd, \
                 tc.tile_pool(name="hdt", bufs=2) as hdt, \
                 tc.tile_pool(name="ps3", bufs=2, space="PSUM") as ps3, \
                 tc.tile_pool(name="ps3z", bufs=1, space="PSUM") as ps3z:

                # ft[n2p, nt, (b4, g64)] fp16 from XBAR-transposed h2r blocks
                ft = hd.tile([128, N2 // 128, 256], F16)
                for hh in range(2):
                    for nt in range(N2 // 128):
                        tmp = hdt.tile([128, 128], F16, tag="t3",
                                       name=f"t3_{hh}_{nt}")
                        nc.sync.dma_start_transpose(
                            tmp[:], h2r[:, hh, 128 * nt:128 * (nt + 1)])
                        for blh in range(2):
                            nc.any.tensor_copy(
                                out=ft[:, nt,
                                       64 * (2 * hh + blh):
                                       64 * (2 * hh + blh + 1)],
                                in_=tmp[:, 64 * blh:64 * (blh + 1)])
                # cch_in rows (r 8, b 4), cols f = (n2p 128, g 64)
                cch_v = cch_in.ap().rearrange("(r b) (p g) -> r p b g",
                                              b=4, p=128)
                for r in range(N2 // 128):
                    nc.sync.dma_start(
                        cch_v[r],
                        ft[:, r, :].rearrange("p (b g) -> p b g", b=4))
                nc.gpsimd.collective_compute(
                    "AllToAll", ALU.bypass, replica_groups=G8,
                    ins=[cch_in.ap()], outs=[cch_out.ap()])

                # fc1: flt[f-part, kt, 32 rb] via XBAR from cch_out
                flt = hd.tile([128, FBLK // 128, B], F16)
                nc.sync.dma_start_transpose(flt[:], cch_out.ap())
                zps = ps3z.tile([32, D], F32)
                for kt in range(FBLK // 128):
                    nc.tensor.matmul(zps[:], flt[:, kt, :], fc1w[:, kt, :],
                                     start=(kt == 0),
                                     stop=(kt == FBLK // 128 - 1))
                zblk = hd.tile([32, D], F32)
                nc.vector.tensor_copy(zblk[:], zps[:])
                nc.sync.dma_start(ccz_in.ap(), zblk[:])
                nc.gpsimd.collective_compute(
                    "AllReduce", ALU.add, replica_groups=G8,
                    ins=[ccz_in.ap()], outs=[ccz_out.ap()])
                zfull = hd.tile([32, D], F32)
                nc.sync.dma_start(zfull[:], ccz_out.ap())
                zb = hd.tile([32, D], F32)
                nc.sync.dma_start(zb[:], fc1b_in.ap())
                nc.vector.tensor_tensor(zfull[:], zfull[:], zb[:], ALU.add)
                zr = hd.tile([32, D], F32)
                nc.scalar.activation(zr[:], zfull[:], ACT.Relu)
                if dbg:
                    nc.sync.dma_start(z_dbg.ap(), zr[:])

                ident = hd.tile([128, 128], F32)
                make_identity(nc, ident[:])
                f2w = hd.tile([128, 4, C], F16)
                nc.sync.dma_start(
                    f2w[:], fc2w_in.ap().rearrange("(t p) c -> p t c", p=128))
                lps = ps3.tile([32, C], F32, tag="lg")
                for t4 in range(4):
                    ztp = ps3.tile([128, 32], F32, tag="zt", name=f"zt{t4}")
                    nc.tensor.transpose(ztp[:], zr[:, 128 * t4:128 * (t4 + 1)],
                                        ident[:32, :32])
                    zts = hdt.tile([128, 32], F16, tag="zts", name=f"zts{t4}")
                    nc.any.tensor_copy(out=zts[:], in_=ztp[:])
                    nc.tensor.matmul(lps[:], zts[:], f2w[:, t4, :],
                                     start=(t4 == 0), stop=(t4 == 3))
                logits = hd.tile([32, C], F32)
                f2b = hd.tile([32, C], F32)
                nc.sync.dma_start(f2b[:], fc2b_in.ap())
                nc.vector.tensor_tensor(logits[:], lps[:], f2b[:], ALU.add)

                mx = hd.tile([32, 1], F32)
                nc.vector.tensor_reduce(mx[:], logits[:], axis=AX.X, op=ALU.max)
                sh = hd.tile([32, C], F32)
                nc.vector.tensor_tensor(sh[:], logits[:],
                                        mx[:].to_broadcast((32, C)),
                                        ALU.subtract)
                ex = hd.tile([32, C], F32)
                nc.scalar.activation(ex[:], sh[:], ACT.Exp)
                sm = hd.tile([32, 1], F32)
                nc.vector.tensor_reduce(sm[:], ex[:], axis=AX.X, op=ALU.add)
                lg = hd.tile([32, 1], F32)
                nc.scalar.activation(lg[:], sm[:], ACT.Ln)
                res = hd.tile([32, C], F32)
                nc.vector.tensor_tensor(res[:], sh[:],
                                        lg[:].to_broadcast((32, C)),
                                        ALU.subtract)
                nc.sync.dma_start(out_t.ap(), res[:])

    nc.compile()
    return nc


def make_inputs(x, edge_index0, edge_index2, W1, b1, W2, b2,
                fc1_w, fc1_b, fc2_w, fc2_b):
    """Build the 8 per-core input maps."""
    A0 = _dense_adj(np.asarray(edge_index0), N0)
    A2 = _dense_adj(np.asarray(edge_index2), N2)
    M1T = _f16((4.0 * (A0 @ A0)).T)        # [N0, N0], col-sliced per core
    A1T = _f16((2.0 * A0).T)
    A2T = _f16((2.0 * A2).T)

    # rank-16 cosine basis and folded W1
    t = np.arange(T)
    C16 = np.cos(2.0 * np.pi * np.outer(t, np.arange(NF)) / T).astype(np.float32)
    W1f = np.asarray(W1, np.float32)       # [K, T, G1]
    Wf = np.zeros((K, NF, G1), np.float32)
    Wf[:, 0] = W1f[:, 0]
    Wf[:, 15] = W1f[:, 15]
    for fp in range(1, 15):
        Wf[:, fp] = W1f[:, fp] + W1f[:, T - fp]

    # c16 stationary [128 rows (bl2 4, t 32), bb2 2, 128 cols (b_loc 8, f 16)]
    c16w = np.zeros((128, 2, 128), np.float32)
    for bb2 in range(2):
        for bl2 in range(4):
            b_loc = 4 * bb2 + bl2
            c16w[32 * bl2:32 * bl2 + T, bb2,
                 16 * b_loc:16 * (b_loc + 1)] = C16
    c16w = _f16(c16w.reshape(128, 256))

    # w1a stationary [128 rows (b_loc 8, f 16), K, bb 2, 128 cols (bl2, g)]
    w1a = np.zeros((128, K, 2, 128), np.float32)
    for bb in range(2):
        for bl2 in range(4):
            b_loc = 4 * bb + bl2
            w1a[16 * b_loc:16 * (b_loc + 1), :, bb,
                32 * bl2:32 * (bl2 + 1)] = Wf.transpose(1, 0, 2)
    w1a = _f16(w1a.reshape(128, K * 2 * 128))

    # w2a stationary [128 rows (bl2 4, g1 32), K, hh 2, 128 cols (blh 2, g2)]
    W2f = np.asarray(W2, np.float32)       # [K, G1, G2]
    w2a = np.zeros((128, K, 2, 128), np.float32)
    for hh in range(2):
        for blh in range(2):
            bl2 = 2 * hh + blh
            w2a[32 * bl2:32 * (bl2 + 1), :, hh,
                64 * blh:64 * (blh + 1)] = W2f.transpose(1, 0, 2)
    w2a = _f16(w2a.reshape(128, K * 2 * 128))

    b1v = np.tile(np.asarray(b1, np.float32), 4).reshape(128, 1)
    b2v = np.tile(np.asarray(b2, np.float32), 2).reshape(128, 1)
    fc1b = np.tile(np.asarray(fc1_b, np.float32)[None, :], (B, 1))
    fc2b = np.tile(np.asarray(fc2_b, np.float32)[None, :], (B, 1))
    fc2w = _f16(np.asarray(fc2_w, np.float32))
    fc1wf = np.asarray(fc1_w, np.float32)
    xf = np.asarray(x, np.float32)         # [B, N0, T]

    ins = []
    for core in range(NCORES):
        h, q = core // 4, core % 4
        # x_cm [128 rows (bl2 4, t 32), bb2 2, 2048 nodes of half h]
        x_cm = np.zeros((128, 2, NH), np.float32)
        for bb2 in range(2):
            for bl2 in range(4):
                bg = 4 * (4 * bb2 + bl2) + q
                x_cm[32 * bl2:32 * bl2 + T, bb2, :] = \
                    xf[bg, NH * h:NH * (h + 1), :].T
        ins.append({
            "x_cm": _f16(x_cm.reshape(128, 2 * NH)),
            "c16w": c16w,
            "m1t": np.ascontiguousarray(M1T[:, NH * h:NH * (h + 1)]),
            "a1t": np.ascontiguousarray(A1T[:, NH * h:NH * (h + 1)]),
            "a2t": A2T,
            "w1a": w1a, "w2a": w2a, "b1v": b1v, "b2v": b2v,
            "fc1w": _f16(fc1wf[FBLK * core:FBLK * (core + 1), :]),
            "fc1b": fc1b, "fc2b": fc2b, "fc2w": fc2w,
        })
    return ins


def batch_perm():
    """flat row order (r, b_loc) -> global batch id."""
    perm = []
    for r in range(NCORES):
        for bl in range(4):
            perm.append(4 * r + bl)
    return np.array(perm)


_CACHED = {}


def kernel(**inputs):
    if "nc" not in _CACHED:
        _CACHED["nc"] = build_program(dbg=False)
    nc = _CACHED["nc"]
    ins = make_inputs(**inputs)
    res = run_bass_kernel_spmd(nc, ins, core_ids=list(range(NCORES)))
    out = np.zeros((B, C), np.float32)
    out[batch_perm()] = res.results[0]["out"]
    return out


# revision 15
# speedup vs baseline: 2.5991x; 1.0519x over previous
"""NetTGCN forward pass on 8 Trainium2 NeuronCores (Bass/Tile).

Key algorithmic move: the reference's real(FFT) along the 30 time taps is a
rank-16 linear map (cos(2*pi*t*f/30) has identical columns for f and 30-f),
so layer 1's Chebyshev recurrence runs on 16 frequency channels per batch
instead of 30 taps - half the spmv FLOPs of a direct fold.

Sharding:
  Layer 1 (4096-node graph): 2-way node-shard x 4-way batch-shard. Per core:
  8 batches x 16 freqs = 128 channels, 2048 own nodes. The state is kept
  CHANNEL-major [128 c, 2048 n]; the spmv is out = state_blk.T @ M^T-rows
  (stationary = node-major state blocks from the gathered DRAM copy, moving =
  SBUF-resident M^T shard, N=512), which directly produces the channel-major
  next state, so the per-k W-contraction needs no transposes. The per-step
  exchange is a 2-rank AllGather (pairs (c, c+4)) of the XBAR-DMA-transposed
  fp16 state (0.5 MB wire, ~16 us), hidden under the other Chebyshev chain's
  spmv (even/odd chains via M = 4*A'^2). fp16 everywhere in layer 1 (states
  included): simulated end-to-end error 1.8e-3.
  Core (h, q) = core h*4+q owns node half h and batches b_loc -> global
  batch 4*b_loc + q; L2 core j owns batches 4j..4j+3.
  Layer 2 (1024-node graph): batch-parallel (core j handles batches
  4j..4j+3 after an 8-rank AllToAll), zero collectives in the loop,
  same channel-major spmv structure, A2 resident, fc1w prefetched meanwhile.
  Head: h2 features redistributed with an 8-rank AllToAll so fc1 is sharded
  over its 65536-row contraction; partial z AllReduced; fc2 + log_softmax
  computed redundantly on every core. Host un-permutes the 32 rows.
"""

import sys

if "/opt/trn_rl_repo" not in sys.path:
    sys.path.insert(0, "/opt/trn_rl_repo")

import numpy as np

import concourse.bacc as bacc
import concourse.mybir as mybir
import concourse.bass_utils as _bu
from concourse.bass_utils import run_bass_kernel_spmd
from concourse.tile import TileContext
from concourse.masks import make_identity

_bu.upload_artifacts = lambda tmpdir: f"file://{tmpdir}"  # no bucket in sandbox

F16 = mybir.dt.float16
F32 = mybir.dt.float32
AX = mybir.AxisListType
ALU = mybir.AluOpType
ACT = mybir.ActivationFunctionType

B, N0, T, K = 32, 4096, 30, 25
G1, G2, D, C = 32, 64, 512, 10
N2 = N0 // 4
NF = 16                 # rank of the real-FFT cosine map
NCORES = 8
NH = N0 // 2            # 2048 own nodes per core (node half)
P2H = N2 // 2           # 512 own pooled nodes
FBLK = (N2 * G2) // NCORES  # 8192 fc1 contraction rows per core

GPAIR = [[0, 4], [1, 5], [2, 6], [3, 7]]
G8 = [list(range(NCORES))]


def _f16(a):
    return np.ascontiguousarray(np.asarray(a, np.float32).astype(np.float16))


def _dense_adj(edge_index, n):
    row = edge_index[0].astype(np.int64)
    col = edge_index[1].astype(np.int64)
    deg = np.zeros(n, np.float32)
    np.add.at(deg, row, 1.0)
    dis = np.where(deg > 0, 1.0 / np.sqrt(np.maximum(deg, 1.0)), 0.0).astype(np.float32)
    w = (-dis[row] * dis[col]).astype(np.float32)
    a = np.zeros((n, n), np.float32)
    np.add.at(a, (row, col), w)
    return a


def build_program(dbg=False):
    nc = bacc.Bacc("TRN2", target_bir_lowering=False, debug=False,
                   num_devices=NCORES)

    x_cm_in = nc.dram_tensor("x_cm", [128, 2 * NH], F16, kind="ExternalInput")
    c16_in = nc.dram_tensor("c16w", [128, 2 * 128], F16, kind="ExternalInput")
    m1t_in = nc.dram_tensor("m1t", [N0, NH], F16, kind="ExternalInput")
    a1t_in = nc.dram_tensor("a1t", [N0, NH], F16, kind="ExternalInput")
    a2t_in = nc.dram_tensor("a2t", [N2, N2], F16, kind="ExternalInput")
    m2t_in = nc.dram_tensor("m2t", [N2, N2], F16, kind="ExternalInput")
    w1_in = nc.dram_tensor("w1a", [128, K * 2 * 128], F16, kind="ExternalInput")
    w2_in = nc.dram_tensor("w2a", [128, K * 2 * 128], F16, kind="ExternalInput")
    b1_in = nc.dram_tensor("b1v", [128, 1], F32, kind="ExternalInput")
    b2_in = nc.dram_tensor("b2v", [128, 1], F32, kind="ExternalInput")
    fc1w_in = nc.dram_tensor("fc1w", [FBLK, D], F16, kind="ExternalInput")
    fc1b_in = nc.dram_tensor("fc1b", [B, D], F32, kind="ExternalInput")
    fc2w_in = nc.dram_tensor("fc2w", [D, C], F16, kind="ExternalInput")
    fc2b_in = nc.dram_tensor("fc2b", [B, C], F32, kind="ExternalInput")

    out_t = nc.dram_tensor("out", [B, C], F32, kind="ExternalOutput")
    if dbg:
        h1_dbg = nc.dram_tensor("h1_dbg", [256, NH], F32, kind="ExternalOutput")
        l2i_dbg = nc.dram_tensor("l2i_dbg", [128, N2], F32, kind="ExternalOutput")
        h2_dbg = nc.dram_tensor("h2_dbg", [256, N2], F32, kind="ExternalOutput")
        ccpo_dbg = nc.dram_tensor("ccpo_dbg", [256, P2H], F16,
                                  kind="ExternalOutput")
        nm0_dbg = nc.dram_tensor("nm0_dbg", [N2, 128], F16,
                                 kind="ExternalOutput")
        t22_dbg = nc.dram_tensor("t22_dbg", [128, N2], F32,
                                 kind="ExternalOutput")
        z_dbg = nc.dram_tensor("z_dbg", [B, D], F32, kind="ExternalOutput")

    cc1_in = [nc.dram_tensor(f"cc1i{i}", [NH, 128], F16) for i in range(2)]
    cc1_out = [nc.dram_tensor(f"cc1o{i}", [N0, 128], F16) for i in range(2)]
    ccp_in = nc.dram_tensor("ccp_in", [256, P2H], F16)
    ccp_out = nc.dram_tensor("ccp_out", [256, P2H], F16)
    cch_in = nc.dram_tensor("cch_in", [NCORES * 4, FBLK], F16)
    cch_out = nc.dram_tensor("cch_out", [NCORES * 4, FBLK], F16)
    ccz_in = nc.dram_tensor("ccz_in", [B, D], F32)
    ccz_out = nc.dram_tensor("ccz_out", [B, D], F32, addr_space="Shared")

    with TileContext(nc) as tc:
        # ======================= LAYER 1 =======================
        with tc.tile_pool(name="l1c", bufs=1) as l1c, \
             tc.tile_pool(name="l1mv", bufs=1) as l1mv, \
             tc.tile_pool(name="l1st", bufs=5) as l1st, \
             tc.tile_pool(name="l1g", bufs=3) as l1g, \
             tc.tile_pool(name="l1nm", bufs=2) as l1nm, \
             tc.tile_pool(name="ps_y", bufs=3, space="PSUM") as ps_y, \
             tc.tile_pool(name="ps_ct", bufs=2, space="PSUM") as ps_ct:

            w1a = l1c.tile([128, K, 2, 128], F16)
            nc.sync.dma_start(
                w1a[:], w1_in.ap().rearrange("p (k b c) -> p k b c", k=K, b=2))
            b1v = l1c.tile([128, 1], F32)
            nc.sync.dma_start(b1v[:], b1_in.ap())
            h1_sb = l1c.tile([128, 2, NH], F32)
            nc.any.memset(h1_sb[:], 0.0)

            # moving-operand buffer: holds a1t for k=1, then m1t for k>=2.
            # Bulk loads ride the scalar-engine HWDGE queue so the
            # latency-critical sync-queue DMAs are not stuck behind them.
            mv = l1mv.tile([128, 32, NH], F16)
            a1_v = a1t_in.ap().rearrange("(t p) n -> t p n", p=128)
            m1_v = m1t_in.ap().rearrange("(t p) n -> t p n", p=128)
            for mb in range(32):
                nc.scalar.dma_start(mv[:, mb, :], a1_v[mb])

            tx = {}

            def xbar_ag(k):
                nm = l1nm.tile([128, NH // 128, 128], F16, tag="nm",
                               name=f"nm{k}")
                nc.sync.dma_start_transpose(nm[:], tx[k][:])
                cin, cout = cc1_in[k % 2], cc1_out[k % 2]
                nc.sync.dma_start(
                    cin.ap().rearrange("(t p) c -> p t c", p=128), nm[:])
                nc.gpsimd.collective_compute(
                    "AllGather", ALU.bypass, replica_groups=GPAIR,
                    ins=[cin.ap()], outs=[cout.ap()])

            def contract(k):
                for bb in range(2):
                    for ns in range(4):
                        cps = ps_ct.tile([128, 512], F32, tag="ct",
                                         name=f"ct{k}_{bb}_{ns}")
                        nc.tensor.matmul(cps[:], w1a[:, k, bb, :],
                                         tx[k][:, 512 * ns:512 * (ns + 1)],
                                         start=True, stop=True)
                        nc.vector.tensor_tensor(
                            h1_sb[:, bb, 512 * ns:512 * (ns + 1)],
                            h1_sb[:, bb, 512 * ns:512 * (ns + 1)],
                            cps[:], ALU.add)

            # ---- x' = x @ C16 (channel-major) ----
            with tc.tile_pool(name="l1x", bufs=1) as l1x:
                c16 = l1x.tile([128, 2, 128], F16)
                nc.sync.dma_start(
                    c16[:], c16_in.ap().rearrange("p (b c) -> p b c", b=2))
                x_v = x_cm_in.ap().rearrange("p (b n) -> p b n", b=2)
                tx[0] = l1st.tile([128, NH], F16, tag="tx", name="tx0")
                for bb2 in range(2):
                    xh = l1x.tile([128, NH], F16, tag="xh", name=f"xh{bb2}")
                    nc.sync.dma_start(xh[:], x_v[:, bb2, :])
                    for ns in range(4):
                        xps = ps_ct.tile([128, 512], F32, tag="ct",
                                         name=f"xp{bb2}_{ns}")
                        nc.tensor.matmul(xps[:], c16[:, bb2, :],
                                         xh[:, 512 * ns:512 * (ns + 1)],
                                         start=True, stop=True)
                        o = tx[0][:, 512 * ns:512 * (ns + 1)]
                        if bb2 == 0:
                            nc.vector.tensor_copy(o, xps[:])
                        else:
                            nc.vector.tensor_tensor(o, o, xps[:], ALU.add)
                xbar_ag(0)

            # ---- Chebyshev steps; contract(k-1) emitted between spmvs ----
            for k in range(1, K):
                gi = 0 if k == 1 else k % 2   # k=1 consumes the x' gather
                gsrc = cc1_out[gi].ap().rearrange("(t p) c -> p t c", p=128)
                tx[k] = l1st.tile([128, NH], F16, tag="tx", name=f"tx{k}")
                stt = []
                for hb in range(2):
                    s = l1g.tile([128, 16, 128], F16, tag="g",
                                 name=f"g{k}_{hb}")
                    nc.sync.dma_start(s[:], gsrc[:, 16 * hb:16 * (hb + 1), :])
                    stt.append(s)
                for half in range(2):
                    yp = ps_y.tile([128, 2, 512], F32, tag="y",
                                   name=f"y{k}_{half}")
                    for mb in range(32):
                        for j in range(2):
                            nc.tensor.matmul(
                                yp[:, j, :], stt[mb // 16][:, mb % 16, :],
                                mv[:, mb, 1024 * half + 512 * j:
                                   1024 * half + 512 * (j + 1)],
                                start=(mb == 0), stop=(mb == 31))
                    o = tx[k][:, 1024 * half:1024 * (half + 1)]
                    ypf = yp[:].rearrange("p a b -> p (a b)")
                    if k == 1:
                        nc.vector.tensor_scalar_mul(o, ypf, 0.5)
                    elif k == 2:
                        nc.vector.tensor_scalar_mul(o, ypf, 0.5)
                        nc.vector.tensor_tensor(
                            o, o, tx[0][:, 1024 * half:1024 * (half + 1)],
                            ALU.subtract)
                    elif k == 3:
                        p1 = tx[1][:, 1024 * half:1024 * (half + 1)]
                        nc.vector.tensor_tensor(o, ypf, p1, ALU.subtract)
                        nc.vector.tensor_tensor(o, o, p1, ALU.subtract)
                        nc.vector.tensor_tensor(o, o, p1, ALU.subtract)
                    else:
                        p2 = tx[k - 2][:, 1024 * half:1024 * (half + 1)]
                        p4 = tx[k - 4][:, 1024 * half:1024 * (half + 1)]
                        nc.vector.tensor_tensor(o, ypf, p2, ALU.subtract)
                        nc.vector.tensor_tensor(o, o, p2, ALU.subtract)
                        nc.vector.tensor_tensor(o, o, p4, ALU.subtract)
                if k == 1:
                    # refill the moving buffer with m1t now that a1t is done
                    for mb in range(32):
                        nc.scalar.dma_start(mv[:, mb, :], m1_v[mb])
                if k < K - 2:
                    xbar_ag(k)
                contract(k - 1)
                tx.pop(k - 4, None)
            contract(K - 1)

            # ---- bias + relu + maxpool4 along nodes ----
            h1p = l1c.tile([128, 2, P2H], F16)
            for bb in range(2):
                nc.scalar.activation(h1_sb[:, bb, :], h1_sb[:, bb, :],
                                     ACT.Relu, bias=b1v[:])
                h4 = h1_sb[:, bb, :].rearrange("p (n f) -> p n f", f=4)
                nc.vector.tensor_tensor(h1p[:, bb, :], h4[:, :, 0],
                                        h4[:, :, 1], ALU.max)
                nc.vector.tensor_tensor(h1p[:, bb, :], h1p[:, bb, :],
                                        h4[:, :, 2], ALU.max)
                nc.vector.tensor_tensor(h1p[:, bb, :], h1p[:, bb, :],
                                        h4[:, :, 3], ALU.max)
            if dbg:
                nc.sync.dma_start(
                    h1_dbg.ap().rearrange("(b p) n -> p b n", p=128), h1_sb[:])

            # 8-rank AllToAll of pooled features. Batch ownership is chosen so
            # slot j (rows 32j..32j+32 = b_loc j's g-rows x own 512 nodes) is
            # exactly what L2 core j needs from this core; the output blocks
            # are then read rank-uniformly.
            nc.sync.dma_start(
                ccp_in.ap().rearrange("(b p) c -> p b c", p=128), h1p[:])
            nc.gpsimd.collective_compute(
                "AllToAll", ALU.bypass, replica_groups=G8,
                ins=[ccp_in.ap()], outs=[ccp_out.ap()])

        # ======================= LAYER 2 =======================
        with tc.tile_pool(name="l2c", bufs=1) as l2c, \
             tc.tile_pool(name="l2st", bufs=5) as l2st, \
             tc.tile_pool(name="l2nm", bufs=3) as l2nm:

            a2t = l2c.tile([128, N2 // 128, N2], F16)
            nc.scalar.dma_start(
                a2t[:], a2t_in.ap().rearrange("(t p) n -> p t n", p=128))
            m2t = l2c.tile([128, N2 // 128, N2], F16)
            nc.scalar.dma_start(
                m2t[:], m2t_in.ap().rearrange("(t p) n -> p t n", p=128))
            w2a = l2c.tile([128, K, 2, 128], F16)
            nc.scalar.dma_start(
                w2a[:], w2_in.ap().rearrange("p (k h c) -> p k h c", k=K, h=2))
            b2v = l2c.tile([128, 1], F32)
            nc.sync.dma_start(b2v[:], b2_in.ap())
            # preload fc1w for the head while layer 2 computes
            fc1w = l2c.tile([128, FBLK // 128, D], F16)
            nc.scalar.dma_start(
                fc1w[:], fc1w_in.ap().rearrange("(t p) d -> p t d", p=128))
            h2_sb = l2c.tile([128, 2, N2], F32)
            nc.any.memset(h2_sb[:], 0.0)

            ident2 = l2c.tile([128, 128], F16)
            make_identity(nc, ident2[:])
            with tc.tile_pool(name="ps2_y", bufs=2, space="PSUM") as ps2_y, \
                 tc.tile_pool(name="ps2_ct", bufs=2, space="PSUM") as ps2_ct, \
                 tc.tile_pool(name="ps2_tr", bufs=2, space="PSUM") as ps2_tr:

                tx2 = {}
                nm2 = {}
                # out block r=(h', q') = core r's slot for me: batch 4*my_j+q'
                # (g1-rows) x n2-half h'
                tx2[0] = l2st.tile([128, N2], F16, tag="tx2", name="tx20")
                for hp in range(2):
                    for qp in range(4):
                        nc.sync.dma_start(
                            tx2[0][32 * qp:32 * (qp + 1),
                                   512 * hp:512 * (hp + 1)],
                            ccp_out.ap()[32 * (4 * hp + qp):
                                         32 * (4 * hp + qp + 1), :])
                if dbg:
                    l2i = l2c.tile([128, N2], F32)
                    nc.vector.tensor_copy(l2i[:], tx2[0][:])
                    nc.sync.dma_start(l2i_dbg.ap(), l2i[:])
                    ccst = l2c.tile([128, 2, P2H], F16)
                    nc.sync.dma_start(
                        ccst[:],
                        ccp_out.ap().rearrange("(a p) c -> p a c", p=128))
                    nc.sync.dma_start(
                        ccpo_dbg.ap().rearrange("(a p) c -> p a c", p=128),
                        ccst[:])

                def xbar2(k):
                    # PE transposes (XBAR->PE edges proved racy on HW)
                    nm2[k] = l2nm.tile([128, N2 // 128, 128], F16, tag="nm2",
                                       name=f"nm2_{k}")
                    for g4 in range(2):
                        trp = ps2_tr.tile([128, 4, 128], F16, tag="tr2",
                                          name=f"tr2_{k}_{g4}")
                        for t in range(4):
                            mb = 4 * g4 + t
                            nc.tensor.transpose(
                                trp[:, t, :],
                                tx2[k][:, 128 * mb:128 * (mb + 1)],
                                ident2[:])
                            nc.any.tensor_copy(out=nm2[k][:, mb, :],
                                               in_=trp[:, t, :])

                def contract2(k):
                    for hh in range(2):
                        for ns in range(2):
                            cps = ps2_ct.tile([128, 512], F32, tag="ct2",
                                              name=f"c2_{k}_{hh}_{ns}")
                            nc.tensor.matmul(
                                cps[:], w2a[:, k, hh, :],
                                tx2[k][:, 512 * ns:512 * (ns + 1)],
                                start=True, stop=True)
                            nc.vector.tensor_tensor(
                                h2_sb[:, hh, 512 * ns:512 * (ns + 1)],
                                h2_sb[:, hh, 512 * ns:512 * (ns + 1)],
                                cps[:], ALU.add)

                # even/odd chains via M2 = 4*A2'^2 (same scheme as layer 1):
                # spmv k consumes nm2[k-2], so the update/XBAR latency of a
                # step hides under the other chain's spmv.
                xbar2(0)
                for k in range(1, K):
                    tx2[k] = l2st.tile([128, N2], F16, tag="tx2",
                                       name=f"tx2{k}")
                    src_nm = nm2[0] if k <= 2 else nm2[k - 2]
                    mvop = a2t if k == 1 else m2t
                    yp = ps2_y.tile([128, 2, 512], F32, tag="y2",
                                    name=f"y2_{k}")
                    for mb in range(N2 // 128):
                        for j in range(2):
                            nc.tensor.matmul(
                                yp[:, j, :], src_nm[:, mb, :],
                                mvop[:, mb, 512 * j:512 * (j + 1)],
                                start=(mb == 0), stop=(mb == N2 // 128 - 1))
                    ypf = yp[:].rearrange("p a b -> p (a b)")
                    if k == 1:
                        nc.vector.tensor_scalar_mul(tx2[1][:], ypf, 0.5)
                    elif k == 2:
                        nc.vector.tensor_scalar_mul(tx2[2][:], ypf, 0.5)
                        nc.vector.tensor_tensor(tx2[2][:], tx2[2][:],
                                                tx2[0][:], ALU.subtract)
                    elif k == 3:
                        nc.vector.tensor_tensor(tx2[3][:], ypf, tx2[1][:],
                                                ALU.subtract)
                        nc.vector.tensor_tensor(tx2[3][:], tx2[3][:],
                                                tx2[1][:], ALU.subtract)
                        nc.vector.tensor_tensor(tx2[3][:], tx2[3][:],
                                                tx2[1][:], ALU.subtract)
                    else:
                        nc.vector.tensor_tensor(tx2[k][:], ypf,
                                                tx2[k - 2][:], ALU.subtract)
                        nc.vector.tensor_tensor(tx2[k][:], tx2[k][:],
                                                tx2[k - 2][:], ALU.subtract)
                        nc.vector.tensor_tensor(tx2[k][:], tx2[k][:],
                                                tx2[k - 4][:], ALU.subtract)
                    if k < K - 2:
                        xbar2(k)
                    contract2(k - 1)
                    if dbg and k == 2:
                        t22 = l2c.tile([128, N2], F32, name="t22")
                        nc.vector.tensor_copy(t22[:], tx2[2][:])
                        nc.sync.dma_start(t22_dbg.ap(), t22[:])
                        nc.sync.dma_start(
                            nm0_dbg.ap().rearrange("(t p) c -> p t c", p=128),
                            nm2[0][:])
                    nm2.pop(k - 4, None)
                    tx2.pop(k - 4, None)
                contract2(K - 1)

                # bias + relu -> fp16 channel-major h2
                h2r = l2c.tile([128, 2, N2], F16)
                for hh in range(2):
                    nc.scalar.activation(h2r[:, hh, :], h2_sb[:, hh, :],
                                         ACT.Relu, bias=b2v[:])
                if dbg:
                    h2f = l2c.tile([128, 2, N2], F32)
                    nc.vector.tensor_copy(h2f[:], h2r[:])
                    nc.sync.dma_start(
                        h2_dbg.ap().rearrange("(h p) n -> p h n", p=128),
                        h2f[:])

            # =================== HEAD ===================
            with tc.tile_pool(name="hd", bufs=1) as hd, \
                 tc.tile_pool(name="hdt", bufs=2) as hdt, \
                 tc.tile_pool(name="ps3", bufs=2, space="PSUM") as ps3, \
                 tc.tile_pool(name="ps3z", bufs=1, space="PSUM") as ps3z:

                ident = hd.tile([128, 128], F16)
                make_identity(nc, ident[:])
                identf = hd.tile([32, 32], F32)
                make_identity(nc, identf[:])
                # ft[n2p, nt, (b4, g64)] fp16 via PE transposes
                ft = hd.tile([128, N2 // 128, 256], F16)
                for hh in range(2):
                    for nt in range(N2 // 128):
                        tmp = ps3.tile([128, 128], F16, tag="zt",
                                       name=f"t3_{hh}_{nt}")
                        nc.tensor.transpose(
                            tmp[:], h2r[:, hh, 128 * nt:128 * (nt + 1)],
                            ident[:])
                        for blh in range(2):
                            nc.any.tensor_copy(
                                out=ft[:, nt,
                                       64 * (2 * hh + blh):
                                       64 * (2 * hh + blh + 1)],
                                in_=tmp[:, 64 * blh:64 * (blh + 1)])
                # cch_in rows (r 8, b 4), cols f = (n2p 128, g 64)
                cch_v = cch_in.ap().rearrange("(r b) (p g) -> r p b g",
                                              b=4, p=128)
                for r in range(N2 // 128):
                    nc.sync.dma_start(
                        cch_v[r],
                        ft[:, r, :].rearrange("p (b g) -> p b g", b=4))
                nc.gpsimd.collective_compute(
                    "AllToAll", ALU.bypass, replica_groups=G8,
                    ins=[cch_in.ap()], outs=[cch_out.ap()])

                # fc1: flt[f-part, kt, 32 rb] via XBAR from cch_out; the
                # XBAR lands in a staging tile and a same-queue DMA copies it,
                # so the PE consumer sees a regular DMA-write edge.
                flt_raw = hd.tile([128, FBLK // 128, B], F16)
                nc.sync.dma_start_transpose(flt_raw[:], cch_out.ap())
                flt = hd.tile([128, FBLK // 128, B], F16)
                nc.sync.dma_start(flt[:], flt_raw[:])
                zps = ps3z.tile([32, D], F32)
                for kt in range(FBLK // 128):
                    nc.tensor.matmul(zps[:], flt[:, kt, :], fc1w[:, kt, :],
                                     start=(kt == 0),
                                     stop=(kt == FBLK // 128 - 1))
                zblk = hd.tile([32, D], F32)
                nc.vector.tensor_copy(zblk[:], zps[:])
                nc.sync.dma_start(ccz_in.ap(), zblk[:])
                nc.gpsimd.collective_compute(
                    "AllReduce", ALU.add, replica_groups=G8,
                    ins=[ccz_in.ap()], outs=[ccz_out.ap()])
                zfull = hd.tile([32, D], F32)
                nc.sync.dma_start(zfull[:], ccz_out.ap())
                zb = hd.tile([32, D], F32)
                nc.sync.dma_start(zb[:], fc1b_in.ap())
                nc.vector.tensor_tensor(zfull[:], zfull[:], zb[:], ALU.add)
                zr = hd.tile([32, D], F32)
                nc.scalar.activation(zr[:], zfull[:], ACT.Relu)
                if dbg:
                    nc.sync.dma_start(z_dbg.ap(), zr[:])

                f2w = hd.tile([128, 4, C], F16)
                nc.sync.dma_start(
                    f2w[:], fc2w_in.ap().rearrange("(t p) c -> p t c", p=128))
                lps = ps3.tile([32, C], F32, tag="lg")
                for t4 in range(4):
                    ztp = ps3.tile([128, 32], F32, tag="zt", name=f"zt{t4}")
                    nc.tensor.transpose(ztp[:], zr[:, 128 * t4:128 * (t4 + 1)],
                                        identf[:])
                    zts = hdt.tile([128, 32], F16, tag="zts", name=f"zts{t4}")
                    nc.any.tensor_copy(out=zts[:], in_=ztp[:])
                    nc.tensor.matmul(lps[:], zts[:], f2w[:, t4, :],
                                     start=(t4 == 0), stop=(t4 == 3))
                logits = hd.tile([32, C], F32)
                f2b = hd.tile([32, C], F32)
                nc.sync.dma_start(f2b[:], fc2b_in.ap())
                nc.vector.tensor_tensor(logits[:], lps[:], f2b[:], ALU.add)

                mx = hd.tile([32, 1], F32)
                nc.vector.tensor_reduce(mx[:], logits[:], axis=AX.X, op=ALU.max)
                sh = hd.tile([32, C], F32)
                nc.vector.tensor_tensor(sh[:], logits[:],
                                        mx[:].to_broadcast((32, C)),
                                        ALU.subtract)
                ex = hd.tile([32, C], F32)
                nc.scalar.activation(ex[:], sh[:], ACT.Exp)
                sm = hd.tile([32, 1], F32)
                nc.vector.tensor_reduce(sm[:], ex[:], axis=AX.X, op=ALU.add)
                lg = hd.tile([32, 1], F32)
                nc.scalar.activation(lg[:], sm[:], ACT.Ln)
                res = hd.tile([32, C], F32)
                nc.vector.tensor_tensor(res[:], sh[:],
                                        lg[:].to_broadcast((32, C)),
                                        ALU.subtract)
                nc.sync.dma_start(out_t.ap(), res[:])

    nc.compile()
    return nc


def make_inputs(x, edge_index0, edge_index2, W1, b1, W2, b2,
                fc1_w, fc1_b, fc2_w, fc2_b):
    """Build the 8 per-core input maps."""
    A0 = _dense_adj(np.asarray(edge_index0), N0)
    A2 = _dense_adj(np.asarray(edge_index2), N2)
    M1T = _f16((4.0 * (A0 @ A0)).T)        # [N0, N0], col-sliced per core
    A1T = _f16((2.0 * A0).T)
    A2T = _f16((2.0 * A2).T)
    M2T = _f16((4.0 * (A2 @ A2)).T)

    # rank-16 cosine basis and folded W1
    t = np.arange(T)
    C16 = np.cos(2.0 * np.pi * np.outer(t, np.arange(NF)) / T).astype(np.float32)
    W1f = np.asarray(W1, np.float32)       # [K, T, G1]
    Wf = np.zeros((K, NF, G1), np.float32)
    Wf[:, 0] = W1f[:, 0]
    Wf[:, 15] = W1f[:, 15]
    for fp in range(1, 15):
        Wf[:, fp] = W1f[:, fp] + W1f[:, T - fp]

    # c16 stationary [128 rows (bl2 4, t 32), bb2 2, 128 cols (b_loc 8, f 16)]
    c16w = np.zeros((128, 2, 128), np.float32)
    for bb2 in range(2):
        for bl2 in range(4):
            b_loc = 4 * bb2 + bl2
            c16w[32 * bl2:32 * bl2 + T, bb2,
                 16 * b_loc:16 * (b_loc + 1)] = C16
    c16w = _f16(c16w.reshape(128, 256))

    # w1a stationary [128 rows (b_loc 8, f 16), K, bb 2, 128 cols (bl2, g)]
    w1a = np.zeros((128, K, 2, 128), np.float32)
    for bb in range(2):
        for bl2 in range(4):
            b_loc = 4 * bb + bl2
            w1a[16 * b_loc:16 * (b_loc + 1), :, bb,
                32 * bl2:32 * (bl2 + 1)] = Wf.transpose(1, 0, 2)
    w1a = _f16(w1a.reshape(128, K * 2 * 128))

    # w2a stationary [128 rows (bl2 4, g1 32), K, hh 2, 128 cols (blh 2, g2)]
    W2f = np.asarray(W2, np.float32)       # [K, G1, G2]
    w2a = np.zeros((128, K, 2, 128), np.float32)
    for hh in range(2):
        for blh in range(2):
            bl2 = 2 * hh + blh
            w2a[32 * bl2:32 * (bl2 + 1), :, hh,
                64 * blh:64 * (blh + 1)] = W2f.transpose(1, 0, 2)
    w2a = _f16(w2a.reshape(128, K * 2 * 128))

    b1v = np.tile(np.asarray(b1, np.float32), 4).reshape(128, 1)
    b2v = np.tile(np.asarray(b2, np.float32), 2).reshape(128, 1)
    fc1b = np.tile(np.asarray(fc1_b, np.float32)[None, :], (B, 1))
    fc2b = np.tile(np.asarray(fc2_b, np.float32)[None, :], (B, 1))
    fc2w = _f16(np.asarray(fc2_w, np.float32))
    fc1wf = np.asarray(fc1_w, np.float32)
    xf = np.asarray(x, np.float32)         # [B, N0, T]

    ins = []
    for core in range(NCORES):
        h, q = core // 4, core % 4
        # x_cm [128 rows (bl2 4, t 32), bb2 2, 2048 nodes of half h]
        x_cm = np.zeros((128, 2, NH), np.float32)
        for bb2 in range(2):
            for bl2 in range(4):
                bg = 4 * (4 * bb2 + bl2) + q
                x_cm[32 * bl2:32 * bl2 + T, bb2, :] = \
                    xf[bg, NH * h:NH * (h + 1), :].T
        ins.append({
            "x_cm": _f16(x_cm.reshape(128, 2 * NH)),
            "c16w": c16w,
            "m1t": np.ascontiguousarray(M1T[:, NH * h:NH * (h + 1)]),
            "a1t": np.ascontiguousarray(A1T[:, NH * h:NH * (h + 1)]),
            "a2t": A2T, "m2t": M2T,
            "w1a": w1a, "w2a": w2a, "b1v": b1v, "b2v": b2v,
            "fc1w": _f16(fc1wf[FBLK * core:FBLK * (core + 1), :]),
            "fc1b": fc1b, "fc2b": fc2b, "fc2w": fc2w,
        })
    return ins


def batch_perm():
    """flat row order (r, b_loc) -> global batch id."""
    perm = []
    for r in range(NCORES):
        for bl in range(4):
            perm.append(4 * r + bl)
    return np.array(perm)


_CACHED = {}


def kernel(**inputs):
    if "nc" not in _CACHED:
        _CACHED["nc"] = build_program(dbg=False)
    nc = _CACHED["nc"]
    ins = make_inputs(**inputs)
    res = run_bass_kernel_spmd(nc, ins, core_ids=list(range(NCORES)))
    out = np.zeros((B, C), np.float32)
    out[batch_perm()] = res.results[0]["out"]
    return out
